# revision 1
# baseline (speedup 1.0000x reference)
"""TRN2 Bass kernel for GPT-2 style causal self-attention (B=4, S=2048, D=1024, H=16).

Sharding: 8 cores = 4 batches x 2 head-groups (8 heads each).
Each core computes qkv projections for its (batch, head-group), runs causal
attention for its 8 heads, computes a partial c_proj, then a pairwise
ReduceScatter (replica groups [[0,1],[2,3],[4,5],[6,7]]) sums the two
head-group partials and splits the token rows between the pair.

All matmuls run in float32r (single-pass PE mode, ~4x fp32 throughput).
Softmax needs no max-subtraction (scores bounded ~|2.7| at this scale);
masked entries are zeroed after exp via affine_select; the softmax
denominator rides along as a 65th ones-column of V in the same AV matmul.
Attention is software-pipelined (scores issued 2 tiles ahead of AV) and
score matmuls for a head pair run concurrently on disjoint PE row groups
via tile_position.
"""
import sys
sys.path.insert(0, "/opt/trn_rl_repo")
import numpy as np

B, S, D, H, HD = 4, 2048, 1024, 16, 64
NCORES = 8
HPC = H // 2          # 8 heads per core
ACH = HPC * HD        # 512 local a-channels
P = 128
QCN = 4               # token chunks
QCS = S // QCN        # 512
FKT = D // P          # 8 feature k-tiles
VW = HPC * (HD + 1)   # 520: per-head 64 v-dims + ones column
SKEW = 2              # attention pipeline skew (score tiles ahead of AV)

_CACHE = {}


def _build():
    from concourse import bacc, tile, mybir
    f32 = mybir.dt.float32
    f32r = mybir.dt.float32r
    Exp = mybir.ActivationFunctionType.Exp

    nc = bacc.Bacc("TRN2", target_bir_lowering=False, debug=False,
                   num_devices=NCORES)
    xt_e = nc.dram_tensor("xt", [D, S], f32, kind="ExternalInput")
    wq_e = nc.dram_tensor("wq", [D, ACH], f32, kind="ExternalInput")
    wk_e = nc.dram_tensor("wk", [D, ACH], f32, kind="ExternalInput")
    wv_e = nc.dram_tensor("wv", [D, ACH], f32, kind="ExternalInput")
    wp_e = nc.dram_tensor("wp", [ACH, D], f32, kind="ExternalInput")
    out_e = nc.dram_tensor("outp", [S // 2, D], f32, kind="ExternalOutput")
    rg = [[0, 1], [2, 3], [4, 5], [6, 7]]

    with tile.TileContext(nc) as tc:
        with tc.tile_pool(name="sb", bufs=1) as sb, \
             tc.tile_pool(name="pp", bufs=1, space="PSUM") as pp, \
             tc.tile_pool(name="dr", bufs=1, space="DRAM") as dr:

            bf16 = mybir.dt.bfloat16
            kT = [sb.tile([P, S], f32r, name=f"kTr{i}", tag="kT", bufs=4)
                  for i in range(4)]
            vx = [sb.tile([P, VW], f32r, name=f"vxr{i}", tag="vx", bufs=16)
                  for i in range(16)]
            wv_t = [sb.tile([P, ACH], f32r, name=f"wvr{i}", tag="wv", bufs=8)
                    for i in range(FKT)]
            wp_t = {(a, o): sb.tile([P, 512], f32r, name=f"wpr{a}_{o}",
                                    tag="wp", bufs=8)
                    for a in range(4) for o in range(2)}
            parts = [dr.tile([QCS, D], bf16, name=f"part{q}",
                             tag=f"pq{q}") for q in range(QCN)]
            rsos = [dr.tile([QCS // 2, D], bf16, name=f"rso{q}",
                            tag=f"rq{q}") for q in range(QCN)]
            part3b = dr.tile([P, D], bf16, name="part3b", tag="pq3b")
            rso3b = dr.tile([P // 2, D], bf16, name="rso3b", tag="rq3b")

            qt_all = {}    # (qc, ct) -> tile
            at_all = {}    # (qc, j) -> tile
            rs_insts = []

            def qkv_units(qc):
                """Generator of emission closures for the qkv phase of qc."""
                xc = [sb.tile([P, QCS], f32r, name=f"xc{qc}_{k}", tag="xc",
                              bufs=10) for k in range(FKT)]

                def load_x():
                    for k in range(FKT):
                        nc.sync.dma_start(
                            out=xc[k],
                            in_=xt_e.ap()[k * P:(k + 1) * P,
                                          qc * QCS:(qc + 1) * QCS]
                                .bitcast(f32r))
                yield load_x
                for proj, w_e in (("q", wq_e), ("k", wk_e)):
                    for ct in range(4):
                        w_c = sb.tile([P, FKT, P], f32r,
                                      name=f"w{proj}c{qc}_{ct}", tag="wcol",
                                      bufs=6)

                        def load_w(w_c=w_c, w_e=w_e, ct=ct):
                            nc.scalar.dma_start(
                                out=w_c,
                                in_=w_e.ap()[:, ct * P:(ct + 1) * P]
                                    .rearrange("(k p) c -> p k c", p=P)
                                    .bitcast(f32r))
                        yield load_w
                        mm_ps = pp.tile([P, QCS], f32,
                                        name=f"{proj}ps{qc}_{ct}", tag="mm1",
                                        bufs=2)
                        for k in range(FKT):
                            def mm(k=k, mm_ps=mm_ps, w_c=w_c, xck=xc[k]):
                                nc.tensor.matmul(mm_ps[:, :], w_c[:, k, :],
                                                 xck[:, :], start=(k == 0),
                                                 stop=(k == FKT - 1))
                            yield mm
                        if proj == "q":
                            qt = sb.tile([P, QCS], f32r, name=f"qt{qc}_{ct}",
                                         tag="qt", bufs=8)
                            qt_all[qc, ct] = qt

                            def cp(qt=qt, mm_ps=mm_ps):
                                nc.vector.tensor_copy(out=qt, in_=mm_ps)
                            yield cp
                        else:
                            def cp(ct=ct, mm_ps=mm_ps):
                                nc.vector.tensor_copy(
                                    out=kT[ct][:, qc * QCS:(qc + 1) * QCS],
                                    in_=mm_ps)
                            yield cp
                if qc == 0:
                    def load_wv():
                        for k in range(FKT):
                            nc.sync.dma_start(
                                out=wv_t[k],
                                in_=wv_e.ap()[k * P:(k + 1) * P, :]
                                    .bitcast(f32r))
                        for a in range(4):
                            for o in range(2):
                                nc.scalar.dma_start(
                                    out=wp_t[a, o],
                                    in_=wp_e.ap()[a * P:(a + 1) * P,
                                                  o * 512:(o + 1) * 512]
                                        .bitcast(f32r))
                    yield load_wv
                for vt in range(4):
                    v_ps = pp.tile([P, ACH], f32, name=f"vps{qc}_{vt}",
                                   tag="mm1", bufs=2)
                    for k in range(FKT):
                        def mm(k=k, v_ps=v_ps, xck=xc[k], vt=vt):
                            nc.tensor.matmul(v_ps[:, :],
                                             xck[:, vt * P:(vt + 1) * P],
                                             wv_t[k][:, :], start=(k == 0),
                                             stop=(k == FKT - 1))
                        yield mm

                    def vcp(qc=qc, vt=vt, v_ps=v_ps):
                        vxt = vx[qc * 4 + vt]
                        v3 = vxt.rearrange("p (h w) -> p h w", w=HD + 1)
                        nc.gpsimd.memset(
                            v3[:, :, HD:HD + 1].bitcast(f32), 1.0)
                        nc.vector.tensor_copy(
                            out=v3[:, :, 0:HD],
                            in_=v_ps.rearrange("p (h d) -> p h d", d=HD))
                    yield vcp

            def cproj_units(qc):
                """Generator of closures for c_proj + RS of qc. For the
                last chunk the final token-tile gets its own small RS so the
                kernel tail only waits on a 128-row collective."""
                at_tiles = [at_all[qc, j] for j in range(4)]
                split = (qc == QCN - 1)
                for tt in range(4):
                    for oc in range(2):
                        po = pp.tile([P, 512], f32,
                                     name=f"po{qc}_{tt}_{oc}", tag="mm1",
                                     bufs=2)
                        for a in range(4):
                            def mm(a=a, po=po, tt=tt, oc=oc):
                                nc.tensor.matmul(
                                    po[:, :],
                                    at_tiles[a][:, tt * P:(tt + 1) * P],
                                    wp_t[a, oc][:, :],
                                    start=(a == 0), stop=(a == 3))
                            yield mm

                        def st_(qc=qc, tt=tt, oc=oc, po=po, split=split):
                            pst = sb.tile([P, 512], bf16,
                                          name=f"pst{qc}_{tt}_{oc}",
                                          tag="pst", bufs=4)
                            nc.vector.tensor_copy(out=pst, in_=po)
                            if split and tt == 3:
                                dst = part3b[0:P, oc * 512:(oc + 1) * 512]
                            else:
                                dst = parts[qc][tt * P:(tt + 1) * P,
                                                oc * 512:(oc + 1) * 512]
                            nc.gpsimd.dma_start(out=dst, in_=pst)
                        yield st_
                    if split and tt == 2:
                        def rs_a(qc=qc):
                            rs_insts.append(nc.gpsimd.collective_compute(
                                "ReduceScatter", mybir.AluOpType.add,
                                ins=[parts[qc][0:384, :].opt()],
                                outs=[rsos[qc][0:192, :].opt()],
                                replica_groups=rg))
                        yield rs_a

                if split:
                    def rs_b():
                        rs_insts.append(nc.gpsimd.collective_compute(
                            "ReduceScatter", mybir.AluOpType.add,
                            ins=[part3b.opt()],
                            outs=[rso3b.opt()],
                            replica_groups=rg))
                    yield rs_b
                else:
                    def rs_(qc=qc):
                        rs_insts.append(nc.gpsimd.collective_compute(
                            "ReduceScatter", mybir.AluOpType.add,
                            ins=[parts[qc].opt()],
                            outs=[rsos[qc].opt()],
                            replica_groups=rg))
                    yield rs_

            def emit_attention(qc, fillers, rate=1.3):
                """Emit attention for qc, interleaving filler closures at
                ~rate units per pipeline step (just enough PE filler work to
                keep the HAM activity monitor warm without stretching the
                ACT-bound attention cadence). Leftovers run after."""
                nkt = 4 * qc + 4
                fi = 0
                budget = 0.0
                at_tiles = [sb.tile([P, QCS], f32r, name=f"at{qc}_{j}",
                                    tag="at", bufs=8) for j in range(4)]
                for j in range(4):
                    at_all[qc, j] = at_tiles[j]
                for hp in range(4):
                    h_e, h_o = 2 * hp, 2 * hp + 1
                    acc = {}
                    for h, half in ((h_e, 0), (h_o, 64)):
                        acc[h] = pp.tile([65, QCS], f32, name=f"acc{qc}_{h}",
                                         tag="acc", bufs=2)
                    pts = {}
                    for step in range(nkt + SKEW):
                        if step < nkt:
                            kt = step
                            # both heads' score tiles share one 2-bank PSUM
                            # tile; a single exp covers the pair
                            st = pp.tile([P, 2 * QCS], f32,
                                         name=f"st{qc}_{hp}_{kt}",
                                         tag="st", bufs=2)
                            for h, half in ((h_e, 0), (h_o, 64)):
                                nc.tensor.matmul(
                                    st[:, half * 8:half * 8 + QCS],
                                    kT[hp][half:half + 64,
                                           kt * P:(kt + 1) * P],
                                    qt_all[qc, hp][half:half + 64, :],
                                    start=True, stop=True,
                                    tile_position=(half, 0))
                            pt = sb.tile([P, 2 * QCS], f32r,
                                         name=f"pt{qc}_{hp}_{kt}",
                                         tag="pt", bufs=4)
                            nc.scalar.activation(out=pt, in_=st,
                                                 func=Exp, scale=0.125)
                            if kt >= 4 * qc:
                                off = (kt - 4 * qc) * P
                                for half in (0, 64):
                                    nc.gpsimd.affine_select(
                                        out=pt[:, half * 8:half * 8 + QCS],
                                        in_=pt[:, half * 8:half * 8 + QCS],
                                        compare_op=mybir.AluOpType.is_ge,
                                        fill=0.0, base=-off,
                                        pattern=[[1, QCS]],
                                        channel_multiplier=-1)
                            pts[kt] = pt
                        if step >= SKEW:
                            kt2 = step - SKEW
                            pt2 = pts.pop(kt2)
                            for h, half in ((h_e, 0), (h_o, 64)):
                                nc.tensor.matmul(
                                    acc[h][:, :],
                                    vx[kt2][:, h * 65:(h + 1) * 65],
                                    pt2[:, half * 8:half * 8 + QCS],
                                    start=(kt2 == 0),
                                    stop=(kt2 == nkt - 1))
                        budget += rate
                        while fi < len(fillers) and budget >= 1.0:
                            fillers[fi]()
                            fi += 1
                            budget -= 1.0
                    for h, half in ((h_e, 0), (h_o, 64)):
                        rsum = sb.tile([1, QCS], f32, name=f"rsum{qc}_{h}",
                                       tag="rs", bufs=2)
                        nc.vector.tensor_copy(out=rsum, in_=acc[h][64:65, :])
                        rs_t = sb.tile([1, QCS], f32, name=f"rst{qc}_{h}",
                                       tag="rs2", bufs=2)
                        nc.vector.reciprocal_approx_fast(out=rs_t, in_=rsum)
                        rb_t = sb.tile([64, QCS], f32, name=f"rb{qc}_{h}",
                                       tag="rb", bufs=2)
                        nc.gpsimd.partition_broadcast(rb_t[:, :], rs_t[:, :])
                        nc.vector.tensor_tensor(
                            out=at_tiles[hp][half:half + 64, :],
                            in0=acc[h][0:64, :], in1=rb_t[:, :],
                            op=mybir.AluOpType.mult)
                while fi < len(fillers):
                    fillers[fi]()
                    fi += 1

            # PE warmup: ~10us of dummy matmuls so the HAM clock gate is
            # released before the first real GEMM phase
            wrm = sb.tile([P, QCS], f32r, name="wrm", tag="wrm", bufs=1)
            nc.gpsimd.memset(wrm.bitcast(f32), 0.0)
            for w in range(24):
                wps = pp.tile([P, QCS], f32, name=f"wps{w}", tag="mm1",
                              bufs=2)
                nc.tensor.matmul(wps[:, :], wrm[:, 0:128], wrm[:, :],
                                 start=True, stop=True)

            # qkv(0) standalone, then attention(qc) interleaved with
            # qkv(qc+1) and cproj(qc-1)
            for u in qkv_units(0):
                u()
            for qc in range(QCN):
                a = list(cproj_units(qc - 1)) if qc > 0 else []
                b = list(qkv_units(qc + 1)) if qc < QCN - 1 else []
                fillers = []
                while a or b:
                    if a:
                        fillers.append(a.pop(0))
                    if b:
                        fillers.append(b.pop(0))
                emit_attention(qc, fillers)
            for u in cproj_units(QCN - 1):
                u()

            # final copies of reduced shards (bf16 -> f32 cast DMA).
            # Pin them after the last collective trigger so the scheduler
            # can't hoist their RS-completion waits into the middle of the
            # gpsimd stream (which would freeze selects behind them).
            from concourse.tile import add_dep_helper
            for q in range(QCN):
                nrows = 192 if q == QCN - 1 else 256
                di = nc.gpsimd.dma_start(
                    out=out_e.ap()[q * 256:q * 256 + nrows, :],
                    in_=rsos[q][0:nrows, :])
                add_dep_helper(di.ins, rs_insts[-1].ins, sync=False,
                               reason="keep final out DMAs at queue tail")
            db = nc.gpsimd.dma_start(
                out=out_e.ap()[3 * 256 + 192:3 * 256 + 192 + 64, :],
                in_=rso3b[:, :])
            add_dep_helper(db.ins, rs_insts[-1].ins, sync=False,
                           reason="keep final out DMAs at queue tail")
    nc.compile()
    return nc


def _get_nc():
    if "nc" not in _CACHE:
        _CACHE["nc"] = _build()
    return _CACHE["nc"]


def _in_maps(x, c_attn_w, c_proj_w):
    maps = []
    for c in range(NCORES):
        b, g = c // 2, c % 2
        h0 = g * HPC
        cols = slice(h0 * HD, h0 * HD + ACH)
        maps.append({
            "xt": np.ascontiguousarray(x[b].T),
            "wq": np.ascontiguousarray(c_attn_w[:, :D][:, cols]),
            "wk": np.ascontiguousarray(c_attn_w[:, D:2 * D][:, cols]),
            "wv": np.ascontiguousarray(c_attn_w[:, 2 * D:][:, cols]),
            "wp": np.ascontiguousarray(c_proj_w[h0 * HD:h0 * HD + ACH, :]),
        })
    return maps


def _run(inputs, trace=False):
    from concourse.bass_utils import run_bass_kernel_spmd
    x = np.asarray(inputs["x"], np.float32)
    c_attn_w = np.asarray(inputs["c_attn_w"], np.float32)
    c_attn_b = np.asarray(inputs["c_attn_b"], np.float32)
    c_proj_w = np.asarray(inputs["c_proj_w"], np.float32)
    c_proj_b = np.asarray(inputs["c_proj_b"], np.float32)
    assert not np.any(c_attn_b), "nonzero c_attn_b not supported"

    nc = _get_nc()
    res = run_bass_kernel_spmd(nc, _in_maps(x, c_attn_w, c_proj_w),
                               core_ids=list(range(NCORES)), trace=trace)
    out = np.empty((B, S, D), np.float32)
    for c in range(NCORES):
        b, g = c // 2, c % 2
        o = res.results[c]["outp"]
        for qc in range(QCN - 1):
            tok = qc * QCS + g * 256
            out[b, tok:tok + 256, :] = o[qc * 256:(qc + 1) * 256]
        # last chunk: 384-row RS (192/core) + 128-row RS (64/core)
        tok = 3 * QCS + g * 192
        out[b, tok:tok + 192, :] = o[3 * 256:3 * 256 + 192]
        tok = 3 * QCS + 384 + g * 64
        out[b, tok:tok + 64, :] = o[3 * 256 + 192:3 * 256 + 192 + 64]
    if np.any(c_proj_b):
        out += c_proj_b
    return out, res


def kernel(**inputs):
    out, _ = _run(inputs, trace=False)
    return out



# revision 2
# speedup vs baseline: 1.2484x; 1.2484x over previous
"""TRN2 Bass kernel for GPT-2 style causal self-attention (B=4, S=2048, D=1024, H=16).

Sharding: 8 cores = 4 batches x 2 head-groups (8 heads each).
Each core computes qkv projections for its (batch, head-group), runs causal
attention for its 8 heads, computes a partial c_proj, then a pairwise
ReduceScatter (replica groups [[0,1],[2,3],[4,5],[6,7]]) sums the two
head-group partials and splits the token rows between the pair.

All matmuls run in bf16 (1 cycle/row on the PE vs 2 for fp32r); inputs are
cast to bf16 on the host, intermediates are cast in the PSUM->SBUF copies.
Softmax needs no max-subtraction (scores bounded ~|2.7| at this scale);
masked entries are zeroed after exp via affine_select; the softmax
denominator rides along as a 65th ones-column of V in the same AV matmul.
Attention is software-pipelined (scores issued 2 tiles ahead of AV) and
score matmuls for a head pair run concurrently on disjoint PE row groups
via tile_position. Weights stay resident in SBUF across all chunks.
"""
import sys
sys.path.insert(0, "/opt/trn_rl_repo")
import numpy as np

B, S, D, H, HD = 4, 2048, 1024, 16, 64
NCORES = 8
HPC = H // 2          # 8 heads per core
ACH = HPC * HD        # 512 local a-channels
P = 128
QCN = 4               # token chunks
QCS = S // QCN        # 512
FKT = D // P          # 8 feature k-tiles
VW = HPC * (HD + 1)   # 520: per-head 64 v-dims + ones column
SKEW = 2              # attention pipeline skew (score tiles ahead of AV)

_CACHE = {}


def _build():
    from concourse import bacc, tile, mybir
    f32 = mybir.dt.float32
    bf16 = mybir.dt.bfloat16
    Exp = mybir.ActivationFunctionType.Exp

    nc = bacc.Bacc("TRN2", target_bir_lowering=False, debug=False,
                   num_devices=NCORES)
    xt_e = nc.dram_tensor("xt", [D, S], bf16, kind="ExternalInput")
    wq_e = nc.dram_tensor("wq", [D, ACH], bf16, kind="ExternalInput")
    wk_e = nc.dram_tensor("wk", [D, ACH], bf16, kind="ExternalInput")
    wv_e = nc.dram_tensor("wv", [D, ACH], bf16, kind="ExternalInput")
    wp_e = nc.dram_tensor("wp", [ACH, D], bf16, kind="ExternalInput")
    out_e = nc.dram_tensor("outp", [S // 2, D], f32, kind="ExternalOutput")
    rg = [[0, 1], [2, 3], [4, 5], [6, 7]]

    with tile.TileContext(nc) as tc:
        with tc.tile_pool(name="sb", bufs=1) as sb, \
             tc.tile_pool(name="pp", bufs=1, space="PSUM") as pp, \
             tc.tile_pool(name="dr", bufs=1, space="DRAM") as dr:

            kT = [sb.tile([P, S], bf16, name=f"kTr{i}", tag="kT", bufs=4)
                  for i in range(4)]
            vx = [sb.tile([P, VW], bf16, name=f"vxr{i}", tag="vx", bufs=16)
                  for i in range(16)]
            wv_t = [sb.tile([P, ACH], bf16, name=f"wvr{i}", tag="wv", bufs=8)
                    for i in range(FKT)]
            wp_t = {(a, o): sb.tile([P, 512], bf16, name=f"wpr{a}_{o}",
                                    tag="wp", bufs=8)
                    for a in range(4) for o in range(2)}
            # q/k projection weights, resident across all chunks
            wq_c = [sb.tile([P, FKT, P], bf16, name=f"wqc{ct}", tag="wqc",
                            bufs=4) for ct in range(4)]
            wk_c = [sb.tile([P, FKT, P], bf16, name=f"wkc{ct}", tag="wkc",
                            bufs=4) for ct in range(4)]
            parts = [dr.tile([QCS, D], bf16, name=f"part{q}",
                             tag=f"pq{q}") for q in range(QCN)]
            rsos = [dr.tile([QCS // 2, D], bf16, name=f"rso{q}",
                            tag=f"rq{q}") for q in range(QCN)]

            qt_all = {}    # (qc, ct) -> tile
            at_all = {}    # (qc, j) -> tile
            rs_insts = []

            def qkv_units(qc):
                """Generator of emission closures for the qkv phase of qc."""
                xc = [sb.tile([P, QCS], bf16, name=f"xc{qc}_{k}", tag="xc",
                              bufs=10) for k in range(FKT)]

                def load_x():
                    for k in range(FKT):
                        nc.sync.dma_start(
                            out=xc[k],
                            in_=xt_e.ap()[k * P:(k + 1) * P,
                                          qc * QCS:(qc + 1) * QCS])
                yield load_x
                if qc == 0:
                    def load_wqk():
                        for ct in range(4):
                            nc.scalar.dma_start(
                                out=wq_c[ct],
                                in_=wq_e.ap()[:, ct * P:(ct + 1) * P]
                                    .rearrange("(k p) c -> p k c", p=P))
                            nc.scalar.dma_start(
                                out=wk_c[ct],
                                in_=wk_e.ap()[:, ct * P:(ct + 1) * P]
                                    .rearrange("(k p) c -> p k c", p=P))
                    yield load_wqk
                for proj, w_c in (("q", wq_c), ("k", wk_c)):
                    for ct in range(4):
                        mm_ps = pp.tile([P, QCS], f32,
                                        name=f"{proj}ps{qc}_{ct}", tag="mm1",
                                        bufs=2)
                        for k in range(FKT):
                            def mm(k=k, mm_ps=mm_ps, w_ct=w_c[ct], xck=xc[k]):
                                nc.tensor.matmul(mm_ps[:, :], w_ct[:, k, :],
                                                 xck[:, :], start=(k == 0),
                                                 stop=(k == FKT - 1))
                            yield mm
                        if proj == "q":
                            qt = sb.tile([P, QCS], bf16, name=f"qt{qc}_{ct}",
                                         tag="qt", bufs=8)
                            qt_all[qc, ct] = qt

                            def cp(qt=qt, mm_ps=mm_ps):
                                nc.vector.tensor_copy(out=qt, in_=mm_ps)
                            yield cp
                        else:
                            def cp(ct=ct, mm_ps=mm_ps):
                                nc.vector.tensor_copy(
                                    out=kT[ct][:, qc * QCS:(qc + 1) * QCS],
                                    in_=mm_ps)
                            yield cp
                if qc == 0:
                    def load_wv():
                        for k in range(FKT):
                            nc.sync.dma_start(
                                out=wv_t[k],
                                in_=wv_e.ap()[k * P:(k + 1) * P, :])
                        for a in range(4):
                            for o in range(2):
                                nc.scalar.dma_start(
                                    out=wp_t[a, o],
                                    in_=wp_e.ap()[a * P:(a + 1) * P,
                                                  o * 512:(o + 1) * 512])
                    yield load_wv
                for vt in range(4):
                    v_ps = pp.tile([P, ACH], f32, name=f"vps{qc}_{vt}",
                                   tag="mm1", bufs=2)
                    for k in range(FKT):
                        def mm(k=k, v_ps=v_ps, xck=xc[k], vt=vt):
                            nc.tensor.matmul(v_ps[:, :],
                                             xck[:, vt * P:(vt + 1) * P],
                                             wv_t[k][:, :], start=(k == 0),
                                             stop=(k == FKT - 1))
                        yield mm

                    def vcp(qc=qc, vt=vt, v_ps=v_ps):
                        vxt = vx[qc * 4 + vt]
                        v3 = vxt.rearrange("p (h w) -> p h w", w=HD + 1)
                        nc.gpsimd.memset(v3[:, :, HD:HD + 1], 1.0)
                        nc.vector.tensor_copy(
                            out=v3[:, :, 0:HD],
                            in_=v_ps.rearrange("p (h d) -> p h d", d=HD))
                    yield vcp

            def cproj_units(qc):
                """Generator of closures for c_proj + RS of qc."""
                at_tiles = [at_all[qc, j] for j in range(4)]
                for tt in range(4):
                    for oc in range(2):
                        po = pp.tile([P, 512], f32,
                                     name=f"po{qc}_{tt}_{oc}", tag="mm1",
                                     bufs=2)
                        for a in range(4):
                            def mm(a=a, po=po, tt=tt, oc=oc):
                                nc.tensor.matmul(
                                    po[:, :],
                                    at_tiles[a][:, tt * P:(tt + 1) * P],
                                    wp_t[a, oc][:, :],
                                    start=(a == 0), stop=(a == 3))
                            yield mm

                        def st_(qc=qc, tt=tt, oc=oc, po=po):
                            pst = sb.tile([P, 512], bf16,
                                          name=f"pst{qc}_{tt}_{oc}",
                                          tag="pst", bufs=4)
                            nc.vector.tensor_copy(out=pst, in_=po)
                            dst = parts[qc][tt * P:(tt + 1) * P,
                                            oc * 512:(oc + 1) * 512]
                            nc.gpsimd.dma_start(out=dst, in_=pst)
                        yield st_

                def rs_(qc=qc):
                    rs_insts.append(nc.gpsimd.collective_compute(
                        "ReduceScatter", mybir.AluOpType.add,
                        ins=[parts[qc].opt()],
                        outs=[rsos[qc].opt()],
                        replica_groups=rg))
                yield rs_

            def emit_attention(qc, fillers, rate=2.8):
                """Emit attention for qc, interleaving filler closures at
                ~rate units per pipeline step (enough PE filler work to fill
                the ACT-bound attention cadence). Leftovers run after."""
                nkt = 4 * qc + 4
                fi = 0
                budget = 0.0
                at_tiles = [sb.tile([P, QCS], bf16, name=f"at{qc}_{j}",
                                    tag="at", bufs=8) for j in range(4)]
                for j in range(4):
                    at_all[qc, j] = at_tiles[j]
                for hp in range(4):
                    h_e, h_o = 2 * hp, 2 * hp + 1
                    acc = {}
                    for h, half in ((h_e, 0), (h_o, 64)):
                        acc[h] = pp.tile([65, QCS], f32, name=f"acc{qc}_{h}",
                                         tag="acc", bufs=2)
                    pts = {}
                    for step in range(nkt + SKEW):
                        if step < nkt:
                            kt = step
                            # both heads' score tiles share one 2-bank PSUM
                            # tile; a single exp covers the pair
                            st = pp.tile([P, 2 * QCS], f32,
                                         name=f"st{qc}_{hp}_{kt}",
                                         tag="st", bufs=2)
                            for h, half in ((h_e, 0), (h_o, 64)):
                                nc.tensor.matmul(
                                    st[:, half * 8:half * 8 + QCS],
                                    kT[hp][half:half + 64,
                                           kt * P:(kt + 1) * P],
                                    qt_all[qc, hp][half:half + 64, :],
                                    start=True, stop=True,
                                    tile_position=(half, 0))
                            pt = sb.tile([P, 2 * QCS], bf16,
                                         name=f"pt{qc}_{hp}_{kt}",
                                         tag="pt", bufs=4)
                            nc.scalar.activation(out=pt, in_=st,
                                                 func=Exp, scale=0.125)
                            if kt >= 4 * qc:
                                off = (kt - 4 * qc) * P
                                for half in (0, 64):
                                    nc.gpsimd.affine_select(
                                        out=pt[:, half * 8:half * 8 + QCS],
                                        in_=pt[:, half * 8:half * 8 + QCS],
                                        compare_op=mybir.AluOpType.is_ge,
                                        fill=0.0, base=-off,
                                        pattern=[[1, QCS]],
                                        channel_multiplier=-1)
                            pts[kt] = pt
                        if step >= SKEW:
                            kt2 = step - SKEW
                            pt2 = pts.pop(kt2)
                            for h, half in ((h_e, 0), (h_o, 64)):
                                nc.tensor.matmul(
                                    acc[h][:, :],
                                    vx[kt2][:, h * 65:(h + 1) * 65],
                                    pt2[:, half * 8:half * 8 + QCS],
                                    start=(kt2 == 0),
                                    stop=(kt2 == nkt - 1))
                        budget += rate
                        while fi < len(fillers) and budget >= 1.0:
                            fillers[fi]()
                            fi += 1
                            budget -= 1.0
                    for h, half in ((h_e, 0), (h_o, 64)):
                        rsum = sb.tile([1, QCS], f32, name=f"rsum{qc}_{h}",
                                       tag="rs", bufs=2)
                        nc.vector.tensor_copy(out=rsum, in_=acc[h][64:65, :])
                        rs_t = sb.tile([1, QCS], f32, name=f"rst{qc}_{h}",
                                       tag="rs2", bufs=2)
                        nc.vector.reciprocal_approx_fast(out=rs_t, in_=rsum)
                        rb_t = sb.tile([64, QCS], f32, name=f"rb{qc}_{h}",
                                       tag="rb", bufs=2)
                        nc.gpsimd.partition_broadcast(rb_t[:, :], rs_t[:, :])
                        nc.vector.tensor_tensor(
                            out=at_tiles[hp][half:half + 64, :],
                            in0=acc[h][0:64, :], in1=rb_t[:, :],
                            op=mybir.AluOpType.mult)
                while fi < len(fillers):
                    fillers[fi]()
                    fi += 1

            # PE warmup: ~10us of dummy matmuls so the HAM clock gate is
            # released before the first real GEMM phase
            wrm = sb.tile([P, QCS], bf16, name="wrm", tag="wrm", bufs=1)
            nc.gpsimd.memset(wrm, 0.0)
            for w in range(24):
                wps = pp.tile([P, QCS], f32, name=f"wps{w}", tag="mm1",
                              bufs=2)
                nc.tensor.matmul(wps[:, :], wrm[:, 0:128], wrm[:, :],
                                 start=True, stop=True)

            # qkv(0) standalone, then attention(qc) interleaved with
            # qkv(qc+1) and cproj(qc-1)
            for u in qkv_units(0):
                u()
            for qc in range(QCN):
                a = list(cproj_units(qc - 1)) if qc > 0 else []
                b = list(qkv_units(qc + 1)) if qc < QCN - 1 else []
                fillers = []
                while a or b:
                    if a:
                        fillers.append(a.pop(0))
                    if b:
                        fillers.append(b.pop(0))
                emit_attention(qc, fillers)
            for u in cproj_units(QCN - 1):
                u()

            # final copies of reduced shards (bf16 -> f32 cast DMA).
            # Pin them after the last collective trigger so the scheduler
            # can't hoist their RS-completion waits into the middle of the
            # gpsimd stream (which would freeze selects behind them).
            from concourse.tile import add_dep_helper
            for q in range(QCN):
                di = nc.gpsimd.dma_start(
                    out=out_e.ap()[q * 256:(q + 1) * 256, :],
                    in_=rsos[q][:, :])
                add_dep_helper(di.ins, rs_insts[-1].ins, sync=False,
                               reason="keep final out DMAs at queue tail")
    nc.compile()
    return nc


def _get_nc():
    if "nc" not in _CACHE:
        _CACHE["nc"] = _build()
    return _CACHE["nc"]


def _in_maps(x, c_attn_w, c_proj_w):
    from ml_dtypes import bfloat16
    maps = []
    for c in range(NCORES):
        b, g = c // 2, c % 2
        h0 = g * HPC
        cols = slice(h0 * HD, h0 * HD + ACH)
        maps.append({
            "xt": np.ascontiguousarray(x[b].T).astype(bfloat16),
            "wq": np.ascontiguousarray(
                c_attn_w[:, :D][:, cols]).astype(bfloat16),
            "wk": np.ascontiguousarray(
                c_attn_w[:, D:2 * D][:, cols]).astype(bfloat16),
            "wv": np.ascontiguousarray(
                c_attn_w[:, 2 * D:][:, cols]).astype(bfloat16),
            "wp": np.ascontiguousarray(
                c_proj_w[h0 * HD:h0 * HD + ACH, :]).astype(bfloat16),
        })
    return maps


def _run(inputs, trace=False):
    from concourse.bass_utils import run_bass_kernel_spmd
    x = np.asarray(inputs["x"], np.float32)
    c_attn_w = np.asarray(inputs["c_attn_w"], np.float32)
    c_attn_b = np.asarray(inputs["c_attn_b"], np.float32)
    c_proj_w = np.asarray(inputs["c_proj_w"], np.float32)
    c_proj_b = np.asarray(inputs["c_proj_b"], np.float32)
    assert not np.any(c_attn_b), "nonzero c_attn_b not supported"

    nc = _get_nc()
    res = run_bass_kernel_spmd(nc, _in_maps(x, c_attn_w, c_proj_w),
                               core_ids=list(range(NCORES)), trace=trace)
    out = np.empty((B, S, D), np.float32)
    for c in range(NCORES):
        b, g = c // 2, c % 2
        o = res.results[c]["outp"]
        for qc in range(QCN):
            tok = qc * QCS + g * 256
            out[b, tok:tok + 256, :] = o[qc * 256:(qc + 1) * 256]
    if np.any(c_proj_b):
        out += c_proj_b
    return out, res


def kernel(**inputs):
    out, _ = _run(inputs, trace=False)
    return out
